# revision 2
# baseline (speedup 1.0000x reference)
"""Trainium2 Bass kernel for nn_ExpertGroup (MoE routing + shared MLP), v2.

Math (per token t, reference semantics):
    h   = silu(x @ W_up.T)                        [T, H]
    a   = h @ W_adapt.T                           [T, A]
    a_e = a @ W_exp_adapters[e].T  (per expert)   [T, E, A]
    sel = a_{last active expert}                  [T, A]
    an  = LayerNorm(sel) * gamma[e] + beta[e]     [T, A]
    out = h @ W_out.T + 0.1 * mask * (an @ W_expert_proj.T) @ W_out.T

Key ideas vs the fp32r v1:
  * Shared path (accuracy-critical) runs in bf16 (same PE rate as fp32r,
    half the weight DMA).  Whole-pipeline bf16 rel err ~3e-3 << 2e-2.
  * Expert path is scaled by 0.1 in the output, so it tolerates fp8:
    all its matmuls run as fp8e4 DoubleRow (2 K-tiles per instruction,
    2x PE throughput).  Fixed power-of-2 scales keep e4m3 in range.
  * W_comb = W_out @ W_expert_proj is computed ON DEVICE once (fp8
    DoubleRow, token-independent), so the expert contribution folds into
    phase 5 as one extra 256-deep matmul per output tile -- the old
    phase-4 h += 0.1*h2 elementwise pass over [H, TL] disappears.
  * LayerNorm is computed on scaled values; since eps dominates var in
    this problem, eps is scaled by the same factor^2 (exact algebra).

Distribution: pure data parallel over tokens, 8 cores x 1024 tokens.
Activations stay feature-major [feature(partition), token(free)]; only
the small LayerNorm/select middle runs token-major via PE transposes.
"""

import sys

sys.path.insert(0, "/opt/trn_rl_repo")

from contextlib import ExitStack

import numpy as np
import ml_dtypes

import concourse.bacc as bacc
import concourse.tile as tile
from concourse import mybir
from concourse.masks import make_identity

# Problem shapes (hardcoded per contest contract)
B, S, D = 4, 2048, 1024
H = 4 * D  # 4096
A = H // 16  # 256
E = 8
NCORES = 8
T = B * S  # 8192
TL = T // NCORES  # 1024 tokens per core
LN_EPS = 1e-5

P = 128
KD = D // P  # 8
KH = H // P  # 32
KA = A // P  # 2
TT = TL // P  # 8 token tiles
NTC = TL // 512  # 2 moving-dim chunks
KHP = KH // 2  # 16 k-tile pairs over H
KDC = D // 512  # 2 D chunks

F32 = mybir.dt.float32
F32R = mybir.dt.float32r
BF16 = mybir.dt.bfloat16
F8 = mybir.dt.float8e4

# fp8 scales (power of 2, ~4-8x margin below e4m3 max 240 on measured absmax)
S_H8 = 2.0**7      # h absmax 0.34 -> 43.6
S_WAD = 2.0**12    # W_adapt absmax 7.6e-3 -> 31.1
S_A = 2.0**11      # a absmax 2.5e-2 -> 51.8
S_WEXP = 2.0**14   # W_exp absmax 3.1e-3 -> 50.9
S_AN = 2.0**12     # 0.1*(LN*g+b) absmax 8.4e-3 -> 34.2
S_WEP = 2.0**14    # W_ep absmax 3.0e-3 -> 48.7
S_WO8 = 2.0**14    # W_out absmax 3.4e-3 -> 55.5
S_WC = 2.0**19     # W_comb absmax 1.2e-4 -> 62.5
P2_DRAIN = S_A / (S_H8 * S_WAD)          # psum(a*S_H8*S_WAD) -> a*S_A
WC_DRAIN = S_WC / (S_WEP * S_WO8)        # psum -> Wc*S_WC
INV_SE = 1.0 / (S_AN * S_WC)             # p5 expert-psum descale
EPS_S = LN_EPS * (S_A * S_WEXP) ** 2     # eps in (scale c3)^2 units


def _build():
    nc = bacc.Bacc("TRN2", target_bir_lowering=False, debug=False)
    ACTF = mybir.ActivationFunctionType
    ALU = mybir.AluOpType
    DR = mybir.MatmulPerfMode.DoubleRow

    x_d = nc.dram_tensor("x_fm", [D, TL], BF16, kind="ExternalInput")
    ew_d = nc.dram_tensor("ew", [TL, E], F32, kind="ExternalInput")
    wup_d = nc.dram_tensor("wup4", [KH, P, KD, P], BF16, kind="ExternalInput")
    wad_d = nc.dram_tensor("wad4", [P, KH, A], F8, kind="ExternalInput")
    wex_d = nc.dram_tensor("wexp4", [P, KA, E, A], F8, kind="ExternalInput")
    wep_d = nc.dram_tensor("wep4", [P, KH, A], F8, kind="ExternalInput")
    wohm_d = nc.dram_tensor("wouthm", [P, KH, D], F8, kind="ExternalInput")
    wout_d = nc.dram_tensor("wout4", [KD, P, KH, P], BF16, kind="ExternalInput")
    gb_d = nc.dram_tensor("gb", [E, 2 * A], F32, kind="ExternalInput")
    out_d = nc.dram_tensor("out_fm", [D, TL], F32, kind="ExternalOutput")

    with tile.TileContext(nc) as tc, ExitStack() as top:
        pers = top.enter_context(tc.tile_pool(name="pers", bufs=1))
        h = pers.tile([P, KH, TL], BF16, name="h")
        S_oh = pers.tile([P, TT, E], F32, name="S_oh")
        nt = pers.tile([P, TT], F32, name="nt")
        eps_t = pers.tile([P, 1], F32, name="eps_t")
        ident = pers.tile([P, P], F32, name="ident")
        nc.vector.memset(eps_t[:, :], EPS_S)
        make_identity(nc, ident[:, :])

        # h8 lives only through phase 2
        h8win = ExitStack()
        h8_p = h8win.enter_context(tc.tile_pool(name="h8p", bufs=1))
        h8 = h8_p.tile([P, KH, TL], F8, name="h8")

        # right-side stack: fp8 weights + small fp8 activations
        w8_p = top.enter_context(tc.tile_pool(name="w8", bufs=1, side="right"))
        wad = w8_p.tile([P, KH, A], F8, name="wad")
        wexp = w8_p.tile([P, KA, E, A], F8, name="wexp")
        a_fm = w8_p.tile([P, KA, TL], F8, name="a_fm")
        an_fm = w8_p.tile([P, KA, TL], F8, name="an_fm")
        wc_fm = w8_p.tile([P, KA, D], F8, name="wc_fm")
        gb01 = w8_p.tile([E, 2 * A], F32R, name="gb01")
        wepwin = ExitStack()
        wep_p = wepwin.enter_context(tc.tile_pool(name="wepp", bufs=1, side="right"))
        wep = wep_p.tile([P, KH, A], F8, name="wep")
        wohm = wep_p.tile([P, KH, D], F8, name="wohm")

        # ---- phase 1: h = silu(x @ W_up.T) in bf16, feature-major ----
        with ExitStack() as p1:
            xp = p1.enter_context(tc.tile_pool(name="xp", bufs=1))
            wup_p = p1.enter_context(tc.tile_pool(name="wup", bufs=8))
            sg_p = p1.enter_context(tc.tile_pool(name="sg", bufs=3))
            pre_p = p1.enter_context(tc.tile_pool(name="pre", bufs=1))
            ps1 = p1.enter_context(tc.tile_pool(name="ps1", bufs=4, space="PSUM"))

            wu_tiles = {}

            def load_wu(key, hb):
                t = wup_p.tile([P, KD, P], BF16, tag="wu", name=f"wu{key}")
                nc.sync.dma_start(out=t[:, :, :], in_=wup_d.ap()[hb])
                wu_tiles[key] = t

            # x split across BOTH queues for max early bandwidth (2KB rows);
            # bulk fp8 weights (needed only ~150us in) queue strictly behind
            load_wu(0, 0)
            x = xp.tile([P, KD, TL], BF16, name="x")
            xr = x_d.ap().rearrange("(kb p) t -> p kb t", p=P)
            for kb in range(4):
                nc.sync.dma_start(out=x[:, kb, :], in_=xr[:, kb, :])
            for kb in range(4, KD):
                nc.gpsimd.dma_start(out=x[:, kb, :], in_=xr[:, kb, :])
            for hb in range(1, 8):
                load_wu(hb, hb)

            # bulk fp8 weights are NOT needed until ~halfway through the
            # kernel; issuing them all at once starves the W_up strip
            # stream (shared DMA engines).  Slice them into ~128KB pieces
            # and pace them out one per p1 chain.
            bulk = [(gb01[:, :], gb_d.ap().bitcast(F32R))]
            for k4 in range(0, KH, 4):
                bulk.append((wad[:, k4 : k4 + 4, :], wad_d.ap()[:, k4 : k4 + 4, :]))
            for ka in range(KA):
                bulk.append((wexp[:, ka, :, :], wex_d.ap()[:, ka, :, :]))
            for k4 in range(0, KH, 4):
                bulk.append((wep[:, k4 : k4 + 4, :], wep_d.ap()[:, k4 : k4 + 4, :]))
            for kh in range(KH):
                bulk.append((wohm[:, kh, :], wohm_d.ap()[:, kh, :]))

            # routing one-hot (last active expert wins), in phase-1's shadow
            ewt = pre_p.tile([P, TT, E], F32, name="ewt")
            nc.gpsimd.dma_start(
                out=ewt[:, :, :], in_=ew_d.ap().rearrange("(tt p) e -> p tt e", p=P)
            )
            act_t = pre_p.tile([P, TT, E], F32, name="act_t")
            nc.vector.tensor_scalar(
                out=act_t[:, :, :], in0=ewt[:, :, :], scalar1=0.0, scalar2=None,
                op0=ALU.is_gt,
            )
            nc.vector.memset(nt[:, :], 1.0)
            for e in range(E - 1, -1, -1):
                nc.vector.tensor_mul(S_oh[:, :, e], act_t[:, :, e], nt[:, :])
                if e:
                    nc.vector.tensor_sub(nt[:, :], nt[:, :], S_oh[:, :, e])

            # hb-major chains (one strip per two chains); the first three
            # hb-blocks run tcx=0 first so the PE starts on partial x
            chains = [(0, 0), (1, 0), (2, 0), (0, 1), (1, 1), (2, 1)]
            for hb in range(3, KH):
                chains += [(hb, 0), (hb, 1)]
            wu_next = 8
            for ci, (hb, tcx) in enumerate(chains):
                wu = wu_tiles[hb]
                sl = slice(tcx * 512, (tcx + 1) * 512)
                ps = ps1.tile([P, 512], F32, tag="ps", name=f"ps1_{hb}_{tcx}")
                for kb in range(KD):
                    nc.tensor.matmul(
                        ps[:, :],
                        wu[:, kb, :],
                        x[:, kb, sl],
                        start=(kb == 0),
                        stop=(kb == KD - 1),
                    )
                if ci % 2 == 1 and wu_next < KH:
                    load_wu(wu_next, wu_next)
                    wu_next += 1
                if ci >= 6 and bulk:
                    dst, src = bulk.pop(0)
                    nc.gpsimd.dma_start(out=dst, in_=src)
                sg = sg_p.tile([P, 512], F32, tag="sg")
                nc.scalar.activation(sg[:, :], ps[:, :], ACTF.Sigmoid)
                nc.vector.tensor_mul(h[:, hb, sl], ps[:, :], sg[:, :])
                nc.scalar.activation(
                    h8[:, hb, sl], h[:, hb, sl], ACTF.Copy, scale=S_H8
                )

        # ---- phases 2+3 interleaved: the tcx=0 half of a = h @ W_adapt.T
        # completes first so the token-major middle (selects + LayerNorm on
        # DVE) starts while the PE still streams p2's tcx=1 half and Wc.

        with ExitStack() as p3:
            ps2 = p3.enter_context(tc.tile_pool(name="ps2", bufs=2, space="PSUM"))
            psc = p3.enter_context(tc.tile_pool(name="psc", bufs=1, space="PSUM"))
            aall_p = p3.enter_context(tc.tile_pool(name="aall", bufs=3, space="PSUM"))
            sm_p = p3.enter_context(tc.tile_pool(name="sm", bufs=2, space="PSUM"))
            asel_p = p3.enter_context(tc.tile_pool(name="asel", bufs=2))
            antm_p = p3.enter_context(tc.tile_pool(name="antm", bufs=TT))
            st_p = p3.enter_context(tc.tile_pool(name="st", bufs=TT))
            stat_p = p3.enter_context(tc.tile_pool(name="stat", bufs=4))
            gbt_p = p3.enter_context(tc.tile_pool(name="gbt", bufs=TT))

            s_ts = {}
            gb_ts = {}
            antms = {}

            # one-hot transposes + gamma/beta gathers for ALL token tiles
            # up front (they only need S_oh/gb01, ready since mid-p1)
            for tt in range(TT):
                pst = sm_p.tile([E, P], F32, tag="sm", name=f"pst{tt}")
                nc.tensor.transpose(pst[:, :], S_oh[:, tt, :], ident[:, :])
                s_t = st_p.tile([E, P], F32R, tag="st", name=f"st{tt}")
                nc.scalar.activation(s_t[:, :], pst[:, :], ACTF.Copy)
                s_ts[tt] = s_t
            for tt in range(TT):
                pg = sm_p.tile([P, 2 * A], F32, tag="sm", name=f"pg{tt}")
                nc.tensor.matmul(
                    pg[:, :], s_ts[tt][:, :], gb01[:, :], start=True, stop=True
                )
                gb_t = gbt_p.tile([P, 2 * A], F32, tag="gbt", name=f"gbt{tt}")
                nc.scalar.activation(gb_t[:, :], pg[:, :], ACTF.Copy)
                gb_ts[tt] = gb_t

            def p2_half(tcx):
                sl = slice(tcx * 512, (tcx + 1) * 512)
                pa = [
                    ps2.tile([P, 512], F32, tag="pa", name=f"pa_{tcx}_{ob}")
                    for ob in range(KA)
                ]
                for k in range(KHP):
                    for ob in range(KA):
                        nc.tensor.matmul(
                            pa[ob][:, :],
                            wad[:, 2 * k : 2 * k + 2, ob * P : (ob + 1) * P],
                            h8[:, 2 * k : 2 * k + 2, sl],
                            start=(k == 0),
                            stop=(k == KHP - 1),
                            perf_mode=DR,
                        )
                for ob in range(KA):
                    nc.scalar.activation(
                        a_fm[:, ob, sl], pa[ob][:, :], ACTF.Copy, scale=P2_DRAIN
                    )

            def middle(tt):
                """adapters (PE) + select (DVE) + LayerNorm (DVE/ACT)."""
                t0 = tt * P
                asel = asel_p.tile([P, A], F32, tag="asel")
                for ep in range(E // 2):
                    pae = aall_p.tile([P, 2 * A], F32, tag="aall")
                    nc.tensor.matmul(
                        pae[:, :],
                        a_fm[:, 0:KA, t0 : t0 + P],
                        wexp[:, 0:KA, 2 * ep : 2 * ep + 2, :],
                        start=True,
                        stop=True,
                        perf_mode=DR,
                    )
                    for half in range(2):
                        e = 2 * ep + half
                        pae_h = pae[:, half * A : (half + 1) * A]
                        if e == 0:
                            nc.vector.tensor_scalar(
                                out=asel[:, :], in0=pae_h,
                                scalar1=S_oh[:, tt, 0:1], scalar2=None,
                                op0=ALU.mult,
                            )
                        else:
                            nc.vector.scalar_tensor_tensor(
                                out=asel[:, :], in0=pae_h,
                                scalar=S_oh[:, tt, e : e + 1], in1=asel[:, :],
                                op0=ALU.mult, op1=ALU.add,
                            )
                st6 = stat_p.tile([P, 6], F32, tag="st6")
                nc.vector.bn_stats(out=st6[:, :], in_=asel[:, :])
                mv = stat_p.tile([P, 2], F32, tag="mv")
                nc.vector.bn_aggr(out=mv[:, :], in_=st6[:, :])
                sq = stat_p.tile([P, 1], F32, tag="sq")
                nc.scalar.activation(sq[:, :], mv[:, 1:2], ACTF.Sqrt, bias=eps_t[:, :])
                rstd = stat_p.tile([P, 1], F32, tag="rstd")
                nc.vector.reciprocal(rstd[:, :], sq[:, :])
                antm = antm_p.tile([P, A], F32, tag="antm", name=f"antm{tt}")
                nc.vector.scalar_tensor_tensor(
                    out=antm[:, :], in0=asel[:, :], scalar=mv[:, 0:1],
                    in1=gb_ts[tt][:, 0:A], op0=ALU.subtract, op1=ALU.mult,
                )
                nc.vector.scalar_tensor_tensor(
                    out=antm[:, :], in0=antm[:, :], scalar=rstd[:, :],
                    in1=gb_ts[tt][:, A : 2 * A], op0=ALU.mult, op1=ALU.add,
                )
                antms[tt] = antm

            def an_trans(tt):
                t0 = tt * P
                for ob in range(KA):
                    ptr = sm_p.tile([P, P], F32, tag="sm", name=f"ptr{tt}_{ob}")
                    nc.tensor.transpose(
                        ptr[:, :], antms[tt][:, ob * P : (ob + 1) * P], ident[:, :]
                    )
                    nc.scalar.activation(
                        an_fm[:, ob, t0 : t0 + P], ptr[:, :], ACTF.Copy
                    )

            p2_half(0)
            for tt in range(TT // 2):
                middle(tt)
            p2_half(1)
            for tt in range(TT // 2, TT):
                middle(tt)

            # W_comb = W_out @ W_ep (fp8 DR) interleaved with the an
            # transposes: the PE streams Wc while DVE finishes LayerNorm
            for c in range(4):
                ab, dc = c // KDC, c % KDC
                pc = psc.tile([P, 512], F32, tag="pc", name=f"pc{ab}_{dc}")
                for k in range(KHP):
                    nc.tensor.matmul(
                        pc[:, :],
                        wep[:, 2 * k : 2 * k + 2, ab * P : (ab + 1) * P],
                        wohm[:, 2 * k : 2 * k + 2, dc * 512 : (dc + 1) * 512],
                        start=(k == 0),
                        stop=(k == KHP - 1),
                        perf_mode=DR,
                    )
                nc.scalar.activation(
                    wc_fm[:, ab, dc * 512 : (dc + 1) * 512],
                    pc[:, :],
                    ACTF.Copy,
                    scale=WC_DRAIN,
                )
                an_trans(2 * c)
                an_trans(2 * c + 1)
        h8win.close()
        wepwin.close()

        # ---- phase 5: out = h @ W_out.T + (an @ Wc) ----
        with ExitStack() as p5:
            wout_p = p5.enter_context(tc.tile_pool(name="wout", bufs=4))
            ob_p = p5.enter_context(tc.tile_pool(name="outsb", bufs=3))
            ps5h = p5.enter_context(tc.tile_pool(name="ps5h", bufs=3, space="PSUM"))
            ps5e = p5.enter_context(tc.tile_pool(name="ps5e", bufs=3, space="PSUM"))
            out_r = out_d.ap().rearrange("(db p) t -> p db t", p=P)

            wo_tiles = {}

            def load_wo(key, db):
                t = wout_p.tile([P, KH, P], BF16, tag="wo", name=f"wo{key}")
                nc.gpsimd.dma_start(out=t[:, :, :], in_=wout_d.ap()[db])
                wo_tiles[key] = t

            load_wo(0, 0)
            load_wo(1, 1)
            for tcx in range(NTC):
                sl = slice(tcx * 512, (tcx + 1) * 512)
                for db in range(KD):
                    key = tcx * KD + db
                    wo = wo_tiles[key]
                    psh = ps5h.tile([P, 512], F32, tag="ph", name=f"ph{key}")
                    for kb in range(KH):
                        nc.tensor.matmul(
                            psh[:, :],
                            wo[:, kb, :],
                            h[:, kb, sl],
                            start=(kb == 0),
                            stop=(kb == KH - 1),
                        )
                    pse = ps5e.tile([P, 512], F32, tag="pe", name=f"pe{key}")
                    nc.tensor.matmul(
                        pse[:, :],
                        wc_fm[:, 0:KA, db * P : (db + 1) * P],
                        an_fm[:, 0:KA, sl],
                        start=True,
                        stop=True,
                        perf_mode=DR,
                    )
                    if key + 2 < 2 * KD:
                        nk = key + 2
                        load_wo(nk, nk % KD)
                    esb = ob_p.tile([P, 512], F32, tag="esb")
                    nc.scalar.activation(
                        esb[:, :], pse[:, :], ACTF.Copy, scale=INV_SE
                    )
                    osb = ob_p.tile([P, 512], F32, tag="osb")
                    nc.vector.tensor_add(osb[:, :], psh[:, :], esb[:, :])
                    nc.sync.dma_start(out=out_r[:, db, sl], in_=osb[:, :])

    nc.compile()
    return nc


_NC = None


def _get_nc():
    global _NC
    if _NC is None:
        _NC = _build()
    return _NC


def _q8(a, s):
    return np.clip(a * s, -240.0, 240.0).astype(ml_dtypes.float8_e4m3)


def _prep_inputs(inputs):
    """Host-side sharding + layout/dtype prep."""
    f = np.float32
    bf = ml_dtypes.bfloat16
    x = np.asarray(inputs["x"], dtype=f).reshape(T, D)
    ew = np.asarray(inputs["expert_weights"], dtype=f).reshape(T, E)
    wup_t = np.asarray(inputs["W_up"], dtype=f).T  # [D, H]
    wad_t = np.asarray(inputs["W_adapt"], dtype=f).T  # [H, A]
    wexp_t = np.asarray(inputs["W_exp_adapters"], dtype=f).transpose(0, 2, 1)  # e,d,o
    gam = np.asarray(inputs["ln_gamma"], dtype=f)
    bet = np.asarray(inputs["ln_beta"], dtype=f)
    wep = np.asarray(inputs["W_expert_proj"], dtype=f)  # [H, A]
    wout_t = np.asarray(inputs["W_out"], dtype=f).T  # [H, D]

    wup4 = np.ascontiguousarray(
        wup_t.reshape(KD, P, KH, P).transpose(2, 1, 0, 3)
    ).astype(bf)
    wad4 = _q8(
        np.ascontiguousarray(wad_t.reshape(KH, P, A).transpose(1, 0, 2)), S_WAD
    )
    wexp4 = _q8(
        np.ascontiguousarray(wexp_t.reshape(E, KA, P, A).transpose(2, 1, 0, 3)),
        S_WEXP,
    )
    wep4 = _q8(np.ascontiguousarray(wep.reshape(KH, P, A).transpose(1, 0, 2)), S_WEP)
    wouthm = _q8(
        np.ascontiguousarray(wout_t.reshape(KH, P, D).transpose(1, 0, 2)), S_WO8
    )
    wout4 = np.ascontiguousarray(
        wout_t.reshape(KH, P, KD, P).transpose(2, 1, 0, 3)
    ).astype(bf)
    gb = np.ascontiguousarray(
        np.concatenate([0.1 * S_AN * gam, 0.1 * S_AN * bet], axis=1)
    ).astype(f)

    shared = {
        "wup4": wup4,
        "wad4": wad4,
        "wexp4": wexp4,
        "wep4": wep4,
        "wouthm": wouthm,
        "wout4": wout4,
        "gb": gb,
    }
    in_maps = []
    for c in range(NCORES):
        sl = slice(c * TL, (c + 1) * TL)
        m = dict(shared)
        m["x_fm"] = np.ascontiguousarray(x[sl].T).astype(bf)  # [D, TL]
        m["ew"] = np.ascontiguousarray(ew[sl])  # [TL, E]
        in_maps.append(m)
    return in_maps


def _gather_output(results):
    outs = [np.asarray(r["out_fm"]).T for r in results]  # each [TL, D]
    return np.ascontiguousarray(np.concatenate(outs, axis=0).reshape(B, S, D))


def _install_trace_shims():
    """Wire up the NTFF profiling hook that this deployment's antenv lacks,
    and stub the artifact-bucket upload (no object store in container)."""
    import types

    import antenv
    from concourse import bass_utils

    try:
        from antenv.axon_hooks import get_axon_ntff_profile_hook  # noqa: F401
    except ImportError:
        sys.path.insert(0, "/root/.axon_site")
        from trn_agent_boot.trn_boot import _ntff_profile_via_ctypes

        hook = _ntff_profile_via_ctypes("/opt/axon/libaxon_pjrt.so")
        mod = types.ModuleType("antenv.axon_hooks")
        mod.get_axon_ntff_profile_hook = lambda: hook
        mod.set_axon_ntff_profile_hook = lambda h: None
        sys.modules["antenv.axon_hooks"] = mod
        antenv.axon_hooks = mod

    bass_utils.upload_artifacts = lambda tmpdir: str(tmpdir)


def run(inputs, trace=False, trace_cores=None):
    """Returns (output, BassKernelResults)."""
    from concourse import bass_utils

    if trace:
        _install_trace_shims()
    nc = _get_nc()
    in_maps = _prep_inputs(inputs)
    res = bass_utils.run_bass_kernel_spmd(
        nc,
        in_maps,
        core_ids=list(range(NCORES)),
        trace=trace,
        trace_cores=trace_cores,
    )
    return _gather_output(res.results), res


def kernel(**inputs) -> np.ndarray:
    out, _ = run(inputs)
    return out
